# revision 1
# baseline (speedup 1.0000x reference)
"""Trainium2 Bass kernel for nn_CDFVarianceLoss.

Math (per sample b, per tensor z in {pred[b], target[b]}, N = 65536):
    z' = (z - min z) / (max z - min z + 1e-6)
    h_j = sum_n exp(-(z'_n - c_j)^2 / (2*sigma^2)) + 1e-6,  c_j = j/63, j < 64
    cdf = cumsum(h / sum_j h)
    loss = mean_{b,j} (cdf_pred[b,j] - cdf_target[b,j])^2

Distribution: data-parallel over the batch — 16 samples over 8 cores,
2 samples per core.  Each core returns the per-(sample, bin) squared CDF
difference [2, 64]; the host averages.

Per-core pipeline:
  - load z natural [128, 512] fp32; DVE per-partition min/max; the
    128-way cross-partition reduction is finished via PE transpose
    (stats -> [2,128] PSUM) + tiny DVE reduces, and the resulting
    (-zmin, 1/(zmax-zmin+eps)) scalars are broadcast back to all 128
    partitions with a ones-column matmul — no DRAM round trips.
  - DVE: z' = (z + (-zmin)) * s (fp32); z'^2 (fp32); then bf16 hi/lo
    splits z' = zhi+zlo, z'^2 = z2hi+z2lo (combined exact to ~2^-17 —
    needed because the exponent is amplified by alpha=200, and bf16
    matmuls stream 4x faster than fp32 on the PE)
  - DMA-reshape the bf16 splits into row tiles [10, CHUNK] whose rows are
    (zhi, zhi, zlo, z2hi, z2lo) x {pred, target}
  - PE bf16 matmul, static block-diag lhsT [10,128] with columns
    [m_hi, m_lo, m_hi, 1, 1] (m = -2c split hi/lo):
    q[j,n] = z'^2 - 2*c_j*z' accumulated exactly in fp32 PSUM
  - ACT: exp(-alpha*q - alpha*c_j^2) with static per-partition bias and
    accum_out -> per-bin partial sums (the only O(N*BINS) pass)
  - DVE reduce -> h; +eps; segmented bin-sums + reciprocal broadcast via
    two tiny matmuls with static 0/1 block matrices
  - PE matmul with static [128,64] cumsum-difference matrix -> cdf diff
  - ACT square -> DMA out
"""

import numpy as np

B = 16
N = 65536
BINS = 64
SIGMA = 0.05
EPS = 1e-6
ALPHA = 0.5 / SIGMA**2  # 200.0
NCORES = 8
SPC = B // NCORES  # samples per core
P = 128
F = N // P  # 512 natural free dim
CHUNK = 16384  # row-layout chunk (elements per rhs row tile)
NCHUNK = N // CHUNK  # 4
MMN = 512  # matmul moving free dim (one PSUM bank of fp32 output)
ACTB = 4  # matmuls per ACT block (PSUM tile = 4 banks)
K = 10  # rhs rows: 5 per tensor x 2 tensors

_CACHE = {}


def _np_bf16_split(x):
    import ml_dtypes

    hi = x.astype(ml_dtypes.bfloat16).astype(np.float32)
    lo = (x - hi).astype(ml_dtypes.bfloat16).astype(np.float32)
    return hi, lo


def _build_nc():
    import concourse.bass as bass
    import concourse.bacc as bacc
    import concourse.tile as tile
    import ml_dtypes
    from concourse import mybir
    from contextlib import ExitStack

    f32 = mybir.dt.float32
    bf16 = mybir.dt.bfloat16
    AX = mybir.AxisListType
    OP = mybir.AluOpType
    ACTF = mybir.ActivationFunctionType

    nc = bacc.Bacc()
    pred_d = nc.declare_dram_parameter("pred", [SPC, N], f32, isOutput=False)
    targ_d = nc.declare_dram_parameter("target", [SPC, N], f32, isOutput=False)
    out_d = nc.declare_dram_parameter("out_sq", [SPC, BINS], f32, isOutput=True)

    c = np.linspace(0.0, 1.0, BINS, dtype=np.float32)
    m_hi, m_lo = _np_bf16_split(-2.0 * c)
    coeffs = np.stack([m_hi, m_lo, m_hi, np.ones(BINS, np.float32),
                       np.ones(BINS, np.float32)])  # [5, 64]
    lhsT_main_np = np.zeros((K, P), np.float32)
    lhsT_main_np[0:5, :BINS] = coeffs
    lhsT_main_np[5:10, BINS:] = coeffs
    lhsT_main_np = lhsT_main_np.astype(ml_dtypes.bfloat16)
    bias_np = np.concatenate([-ALPHA * c * c, -ALPHA * c * c]).reshape(P, 1)
    bias_np = bias_np.astype(np.float32)
    # cumsum-and-subtract: out[m] = sum_{k<=m} hn_pred[k] - sum_{k<=m} hn_targ[k]
    lhsT_tail_np = np.zeros((P, BINS), np.float32)
    for mcol in range(BINS):
        lhsT_tail_np[: mcol + 1, mcol] = 1.0
        lhsT_tail_np[BINS : BINS + mcol + 1, mcol] = -1.0
    # segmented-sum / segmented-broadcast 0/1 blocks
    blk_np = np.zeros((P, 2), np.float32)
    blk_np[:BINS, 0] = 1.0
    blk_np[BINS:, 1] = 1.0
    ones_row_np = np.ones((1, P), np.float32)
    ident_np = np.eye(P, dtype=np.float32)

    lhsT_main_d = nc.inline_tensor(lhsT_main_np, name="lhsT_main")
    bias_d = nc.inline_tensor(bias_np, name="bias_col")
    lhsT_tail_d = nc.inline_tensor(lhsT_tail_np, name="lhsT_tail")
    blk_d = nc.inline_tensor(blk_np, name="blk")
    blkT_d = nc.inline_tensor(np.ascontiguousarray(blk_np.T), name="blkT")
    ones_d = nc.inline_tensor(ones_row_np, name="ones_row")
    ident_d = nc.inline_tensor(ident_np, name="ident")

    with tile.TileContext(nc) as tc, ExitStack() as ctx:
        singles = ctx.enter_context(tc.tile_pool(name="singles", bufs=1))
        nat = ctx.enter_context(tc.tile_pool(name="nat", bufs=2))
        norm = ctx.enter_context(tc.tile_pool(name="norm", bufs=2))
        small = ctx.enter_context(tc.tile_pool(name="small", bufs=2))
        rows = ctx.enter_context(tc.tile_pool(name="rows", bufs=3))
        scr = ctx.enter_context(tc.tile_pool(name="scr", bufs=2))
        hp = ctx.enter_context(tc.tile_pool(name="hp", bufs=2))
        ps_pool = ctx.enter_context(tc.tile_pool(name="ps", bufs=2, space="PSUM"))
        st_pool = ps_pool

        # DMA queue roles: sync carries the steady-state reshape stream;
        # gpsimd carries loads/consts/outputs so they never sit behind a
        # dependency-blocked reshape (in-order queues).
        def dma_ld(out, in_):
            nc.gpsimd.dma_start(out=out, in_=in_)

        def dma_rs(out, in_):
            nc.sync.dma_start(out=out, in_=in_)

        lhsT_main_sb = singles.tile([K, P], bf16)
        dma_ld(lhsT_main_sb, lhsT_main_d[:, :])
        bias_sb = singles.tile([P, 1], f32)
        dma_ld(bias_sb, bias_d[:, :])
        lhsT_tail_sb = singles.tile([P, BINS], f32)
        dma_ld(lhsT_tail_sb, lhsT_tail_d[:, :])
        blk_sb = singles.tile([P, 2], f32)
        dma_ld(blk_sb, blk_d[:, :])
        blkT_sb = singles.tile([2, P], f32)
        dma_ld(blkT_sb, blkT_d[:, :])
        ones_sb = singles.tile([1, P], f32)
        dma_ld(ones_sb, ones_d[:, :])
        ident_sb = singles.tile([P, P], f32)
        dma_ld(ident_sb, ident_d[:, :])

        def load_and_norm(p):
            zA = nat.tile([P, F], f32, tag="zA")
            dma_ld(zA, pred_d[p, :].rearrange("(p f) -> p f", p=P))
            zB = nat.tile([P, F], f32, tag="zB")
            dma_ld(zB, targ_d[p, :].rearrange("(p f) -> p f", p=P))

            def norm_one(z, tag):
                # per-partition (min, -max) over the free dim
                mm = small.tile([P, 2], f32, tag=f"mm{tag}")
                nc.vector.tensor_reduce(out=mm[:, 0:1], in_=z, axis=AX.X, op=OP.min)
                nc.vector.tensor_reduce(
                    out=mm[:, 1:2], in_=z, axis=AX.X, op=OP.max, negate=True
                )
                # finish the cross-partition reduction via PE transpose:
                # one min-reduce of [2,128] gives (zmin, -zmax)
                t1p = st_pool.tile([2, P], f32, tag="ps")
                nc.tensor.transpose(t1p, mm, ident_sb[:, :])
                t1 = small.tile([2, P], f32, tag=f"t1{tag}")
                nc.vector.tensor_copy(t1, t1p)
                mn2 = small.tile([2, 1], f32, tag=f"mn2{tag}")
                nc.vector.tensor_reduce(out=mn2, in_=t1, axis=AX.X, op=OP.min)
                # gather (zmin, -zmax) onto partition 0
                t2p = st_pool.tile([1, 2], f32, tag="ps")
                nc.tensor.transpose(t2p, mn2, ident_sb[0:2, 0:2])
                t2 = small.tile([1, 2], f32, tag=f"t2{tag}")
                nc.vector.tensor_copy(t2, t2p)
                # sc = [-zmin, 1/(zmax - zmin + eps)] on partition 0
                sc = small.tile([1, 2], f32, tag=f"sc{tag}")
                nc.vector.tensor_scalar_mul(sc[0:1, 0:1], t2[0:1, 0:1], -1.0)
                r = small.tile([1, 1], f32, tag=f"r{tag}")
                # r = -((-zmax) + zmin) + eps = zmax - zmin + eps
                nc.vector.tensor_scalar(
                    r, t2[0:1, 1:2], t2[0:1, 0:1], -1.0, OP.add, OP.mult
                )
                nc.vector.tensor_scalar_add(r, r, EPS)
                nc.vector.reciprocal(sc[0:1, 1:2], r)
                # broadcast to all partitions with a ones-column matmul
                nbp = st_pool.tile([P, 2], f32, tag="ps")
                nc.tensor.matmul(nbp, ones_sb[:, :], sc, start=True, stop=True)
                nb = small.tile([P, 2], f32, tag=f"nb{tag}")
                nc.vector.tensor_copy(nb, nbp)
                zp = norm.tile([P, F], f32, tag=f"zp{tag}")
                nc.vector.tensor_scalar(zp, z, nb[:, 0:1], nb[:, 1:2], OP.add, OP.mult)
                zp2 = norm.tile([P, F], f32, tag=f"zp2{tag}")
                nc.vector.tensor_mul(zp2, zp, zp)
                # bf16 hi/lo splits (combined exact to ~2^-17)
                zhi = norm.tile([P, F], bf16, tag=f"zhi{tag}")
                nc.vector.tensor_copy(zhi, zp)
                zlo = norm.tile([P, F], bf16, tag=f"zlo{tag}")
                nc.vector.tensor_sub(zlo, zp, zhi)
                z2hi = norm.tile([P, F], bf16, tag=f"z2hi{tag}")
                nc.vector.tensor_copy(z2hi, zp2)
                z2lo = norm.tile([P, F], bf16, tag=f"z2lo{tag}")
                nc.vector.tensor_sub(z2lo, zp2, z2hi)
                return zhi, zlo, z2hi, z2lo

            rowsA = norm_one(zA, "A")
            rowsB = norm_one(zB, "B")
            # rhs row order must match lhsT_main rows
            return [rowsA[0], rowsA[0], rowsA[1], rowsA[2], rowsA[3],
                    rowsB[0], rowsB[0], rowsB[1], rowsB[2], rowsB[3]]

        mm_per_chunk = CHUNK // MMN  # 32
        blocks = []  # list of (start_mm, n_mm) per ACT block
        i = 0
        while i < mm_per_chunk:
            n = min(ACTB, mm_per_chunk - i)
            blocks.append((i, n))
            i += n
        pp = CHUNK // F  # natural partitions per chunk (32)

        srcs_p = [load_and_norm(p) for p in range(SPC)]
        hparts_p = []
        for p in range(SPC):
            hparts_t = hp.tile(
                [P, NCHUNK * len(blocks)], f32, tag=f"hparts{p}", name=f"hparts{p}"
            )
            hparts_p.append(hparts_t)
        # interleave the two samples' chunk pipelines so the ACT stream
        # stays dense across the whole kernel (no pair-boundary stall)
        for ch in range(NCHUNK):
            for p in range(SPC):
                srcs = srcs_p[p]
                hparts = hparts_p[p]
                rt = rows.tile([K, CHUNK], bf16, tag="rt")
                sl = slice(ch * pp, (ch + 1) * pp)
                for r, src in enumerate(srcs):
                    dma_rs(rt[r : r + 1, :], src[sl, :])
                for bi, (mm0, nmm) in enumerate(blocks):
                    ps = ps_pool.tile([P, ACTB * MMN], f32, tag="ps")
                    for k in range(nmm):
                        col = (mm0 + k) * MMN
                        nc.tensor.matmul(
                            ps[:, k * MMN : (k + 1) * MMN],
                            lhsT_main_sb[:, :],
                            rt[:, col : col + MMN],
                            start=True,
                            stop=True,
                        )
                    sc_t = scr.tile([P, ACTB * MMN], f32, tag="sc")
                    icol = ch * len(blocks) + bi
                    nc.scalar.activation(
                        out=sc_t[:, : nmm * MMN],
                        in_=ps[:, : nmm * MMN],
                        func=ACTF.Exp,
                        bias=bias_sb[:, 0:1],
                        scale=-ALPHA,
                        accum_out=hparts[:, icol : icol + 1],
                    )

        for p in range(SPC):
            hparts = hparts_p[p]
            hcol = small.tile([P, 1], f32, tag="hcol")
            nc.vector.tensor_reduce(out=hcol, in_=hparts, axis=AX.X, op=OP.add)
            heps = small.tile([P, 1], f32, tag="heps")
            nc.vector.tensor_scalar_add(heps, hcol, EPS)
            # segmented sums over the two 64-bin halves via 0/1 matmul,
            # reciprocal, then segmented broadcast via the transposed block
            s2p = st_pool.tile([2, 1], f32, tag="ps")
            nc.tensor.matmul(s2p, blk_sb[:, :], heps, start=True, stop=True)
            sinv2 = small.tile([2, 1], f32, tag="sinv2")
            nc.vector.reciprocal(sinv2, s2p)
            sbp = st_pool.tile([P, 1], f32, tag="ps")
            nc.tensor.matmul(sbp, blkT_sb[:, :], sinv2, start=True, stop=True)
            sinv = small.tile([P, 1], f32, tag="sinv")
            nc.vector.tensor_copy(sinv, sbp)
            hn = small.tile([P, 1], f32, tag="hn")
            nc.vector.tensor_mul(hn, heps, sinv)
            pst = st_pool.tile([BINS, 1], f32, tag="ps")
            nc.tensor.matmul(pst, lhsT_tail_sb[:, :], hn, start=True, stop=True)
            sq = small.tile([BINS, 1], f32, tag="sq")
            nc.scalar.square(sq, pst)
            dma_ld(out_d[p, :], sq[:, 0:1])

    nc.compile()
    return nc


def kernel(pred: np.ndarray, target: np.ndarray) -> np.ndarray:
    from concourse.bass_utils import run_bass_kernel_spmd

    if "nc" not in _CACHE:
        _CACHE["nc"] = _build_nc()
    nc = _CACHE["nc"]

    pred = np.ascontiguousarray(np.asarray(pred, np.float32).reshape(B, N))
    target = np.ascontiguousarray(np.asarray(target, np.float32).reshape(B, N))
    in_maps = [
        {
            "pred": pred[i * SPC : (i + 1) * SPC],
            "target": target[i * SPC : (i + 1) * SPC],
        }
        for i in range(NCORES)
    ]
    res = run_bass_kernel_spmd(nc, in_maps, list(range(NCORES)))
    sq = np.concatenate([r["out_sq"] for r in res.results], axis=0)  # [16, 64]
    return np.float32(np.mean(sq, dtype=np.float64))



# revision 6
# speedup vs baseline: 3.3227x; 3.3227x over previous
"""Trainium2 Bass kernel for nn_CDFVarianceLoss.

Math (per sample b, per tensor z in {pred[b], target[b]}, N = 65536):
    z' = (z - min z) / (max z - min z + 1e-6)
    h_j = sum_n exp(-(z'_n - c_j)^2 / (2*sigma^2)) + 1e-6,  c_j = j/63, j < 64
    cdf = cumsum(h / sum_j h)
    loss = mean_{b,j} (cdf_pred[b,j] - cdf_target[b,j])^2

Key identity: the cumulative kernel sums S_j = sum_n V_j(z'_n) with
V_j(z) = sum_{k<=j} exp(-alpha (z-c_k)^2) are smooth sigmoid-like functions
of z.  Each V_j is approximated (offline least squares, rel error on the
loss ~1e-4) by a linear combination of M=8 shifted erfs plus a constant:
    V_j(z) ~= sum_k R[j,k] erf((g_k - z)/s2) + R[j,M]
so the device only computes the M basis sums D_k = sum_n erf((g_k-z')/s2)
per array -- M activation passes over the natural [128, 512] layout with
scalar scale/bias (the erf argument is affine in z) -- and the host applies
R in float64.  This removes the O(N*BINS) exp/matmul grids entirely:
ACT does N*M work, nothing else is hot.

Distribution: data-parallel over the batch -- 16 samples over 8 cores,
2 samples per core (4 arrays of 65536 per core: pred/target x 2 samples).

Per-core pipeline:
  - load 4 arrays natural [128, 512] fp32 into one [128, 2048] tile
  - per-array min/max: DVE free-dim reduce + gpsimd cross-partition reduce
  - fixups on partition 0 -> (-zmin, 1/(zmax-zmin+eps)) x 4; PE ones-matmul
    broadcasts them to all partitions; DVE tensor_scalar normalizes
  - M x ACT Erf passes [128, 2048] (all 4 arrays at once), scalar
    scale=-1/s2, bias=g_k/s2, output fp32 to rotating SBUF tiles
  - per-pass segmented reduce [128, 4, 512] -> [128, 4]: most on DVE,
    the last passes on gpsimd (XYZWC direct to partial scalars) to balance
  - gpsimd cross-partition reduce -> [1, 4*M] basis sums -> DMA out
  - host: D @ R^T in fp64, then the exact eps/normalize/cumsum/mse tail
"""

import math

import numpy as np

B = 16
N = 65536
BINS = 64
SIGMA = 0.05
EPS = 1e-6
ALPHA = 0.5 / SIGMA**2  # 200.0
NCORES = 8
SPC = B // NCORES  # samples per core
NARR = 2 * SPC  # arrays per core: (pred, target) x samples
P = 128
F = N // P  # 512 natural free dim

# erf basis (designed offline; see module docstring)
M = 8
G_LO, G_HI = -0.14, 1.14
SIG_FIT = 0.14
S2 = SIG_FIT * math.sqrt(2.0)
G_PTS = [G_LO + (G_HI - G_LO) * k / (M - 1) for k in range(M)]
# which erf passes reduce on gpsimd instead of DVE (engine balance)
GP_KS = (6, 7)

_CACHE = {}


def _fit_R():
    """Least-squares fit of V_j(z) in the erf basis + constant (fp64)."""
    nz = 40001
    zg = np.linspace(0.0, 1.0, nz)
    c = np.linspace(0.0, 1.0, BINS)
    K = np.exp(-ALPHA * (zg[None, :] - c[:, None]) ** 2)
    Vt = np.cumsum(K, axis=0)  # [64, nz]
    g = np.asarray(G_PTS)
    erf_v = np.vectorize(math.erf)
    Fb = np.concatenate(
        [erf_v((g[:, None] - zg[None, :]) / S2), np.ones((1, nz))], axis=0
    )  # [M+1, nz]
    w = np.ones(nz)
    w[zg < 0.02] = 3.0
    w[zg > 0.98] = 3.0
    Aw = Fb * w[None, :]
    Gm = Aw @ Fb.T
    Rhs = (Vt * w[None, :]) @ Fb.T
    return np.linalg.solve(Gm + 1e-10 * np.eye(M + 1), Rhs.T).T  # [64, M+1]


def _build_nc():
    import concourse.bass as bass  # noqa: F401
    import concourse.bacc as bacc
    import concourse.tile as tile
    from concourse import mybir
    from contextlib import ExitStack

    f32 = mybir.dt.float32
    AX = mybir.AxisListType
    OP = mybir.AluOpType
    ACTF = mybir.ActivationFunctionType

    nc = bacc.Bacc()
    pred_d = nc.declare_dram_parameter("pred", [SPC, N], f32, isOutput=False)
    targ_d = nc.declare_dram_parameter("target", [SPC, N], f32, isOutput=False)
    out_d = nc.declare_dram_parameter("dsums", [1, NARR * M], f32, isOutput=True)

    ones_row_np = np.ones((1, P), np.float32)
    bias_np = np.broadcast_to(
        np.asarray([g / S2 for g in G_PTS], np.float32)[None, :], (P, M)
    ).copy()

    ones_d = nc.inline_tensor(ones_row_np, name="ones_row")
    bias_d = nc.inline_tensor(bias_np, name="erf_bias")

    with tile.TileContext(nc) as tc, ExitStack() as ctx:
        singles = ctx.enter_context(tc.tile_pool(name="singles", bufs=1))
        nat = ctx.enter_context(tc.tile_pool(name="nat", bufs=1))
        eo_pool = ctx.enter_context(tc.tile_pool(name="eo", bufs=3))
        small = ctx.enter_context(tc.tile_pool(name="small", bufs=2))
        ps_pool = ctx.enter_context(tc.tile_pool(name="ps", bufs=2, space="PSUM"))

        ones_sb = singles.tile([1, P], f32)
        nc.sync.dma_start(out=ones_sb, in_=ones_d[:, :])
        bias_sb = singles.tile([P, M], f32)
        nc.sync.dma_start(out=bias_sb, in_=bias_d[:, :])

        # natural input, all arrays side by side: [128, 4*512]
        zn = nat.tile([P, NARR * F], f32, name="zn")
        srcs = [pred_d[0, :], targ_d[0, :], pred_d[1, :], targ_d[1, :]]
        for a, src in enumerate(srcs):
            nc.sync.dma_start(
                out=zn[:, a * F : (a + 1) * F],
                in_=src.rearrange("(p f) -> p f", p=P),
            )

        # per-array (-min, max) -> mn_all [1, 8] on partition 0 (cross-lane
        # gpsimd reduce only supports add/avg/max, so carry -min and use max)
        mn_all = small.tile([1, 2 * NARR], f32, tag="mn")
        for a in range(NARR):
            sl = zn[:, a * F : (a + 1) * F]
            mm = small.tile([P, 2], f32, tag=f"mm{a}")
            nc.vector.tensor_reduce(
                out=mm[:, 0:1], in_=sl, axis=AX.X, op=OP.min, negate=True
            )
            nc.vector.tensor_reduce(out=mm[:, 1:2], in_=sl, axis=AX.X, op=OP.max)
            # cross-partition finish on gpsimd: (-zmin_a, zmax_a)
            nc.gpsimd.tensor_reduce(
                out=mn_all[0:1, 2 * a : 2 * a + 2], in_=mm, axis=AX.C, op=OP.max
            )

        # fixups on partition 0: nbsrc = [-zmin_a x4 | 1/(zmax-zmin+eps) x4]
        nbsrc = small.tile([1, 2 * NARR], f32, tag="nbsrc")
        rng = small.tile([1, NARR], f32, tag="rng")
        # rng_a = (-zmin_a) + zmax_a + eps
        nc.vector.tensor_tensor(
            out=rng,
            in0=mn_all[0:1, 0 : 2 * NARR : 2],
            in1=mn_all[0:1, 1 : 2 * NARR : 2],
            op=OP.add,
        )
        nc.vector.tensor_scalar_add(rng, rng, EPS)
        nc.vector.reciprocal(nbsrc[0:1, NARR : 2 * NARR], rng)
        nc.vector.tensor_copy(
            nbsrc[0:1, 0:NARR], mn_all[0:1, 0 : 2 * NARR : 2]
        )

        # broadcast to all partitions with a ones-column matmul
        nbp = ps_pool.tile([P, 2 * NARR], f32, tag="ps")
        nc.tensor.matmul(nbp, ones_sb[:, :], nbsrc, start=True, stop=True)
        nb = small.tile([P, 2 * NARR], f32, tag="nb")
        nc.vector.tensor_copy(nb, nbp)

        # normalize each array in place -> z' in [0, 1]
        zc = nat.tile([P, NARR * F], f32, name="zc")
        for a in range(NARR):
            nc.vector.tensor_scalar(
                zc[:, a * F : (a + 1) * F],
                zn[:, a * F : (a + 1) * F],
                nb[:, a : a + 1],
                nb[:, NARR + a : NARR + a + 1],
                OP.add,
                OP.mult,
            )

        # erf spine + segmented reduces
        red = small.tile([P, len(GP_KS) and (M - len(GP_KS)) * NARR or M * NARR],
                         f32, tag="red", name="red")
        outsb = small.tile([1, NARR * M], f32, tag="outsb", name="outsb")
        dve_col = 0
        for k in range(M):
            eo = eo_pool.tile([P, NARR * F], f32, tag="eo")
            nc.scalar.activation(
                out=eo,
                in_=zc,
                func=ACTF.Erf,
                bias=bias_sb[:, k : k + 1],
                scale=float(-1.0 / S2),
            )
            if k in GP_KS:
                for a in range(NARR):
                    nc.gpsimd.tensor_reduce(
                        out=outsb[0:1, (k * NARR + a) : (k * NARR + a + 1)],
                        in_=eo[:, a * F : (a + 1) * F],
                        axis=AX.XYZWC,
                        op=OP.add,
                    )
            else:
                nc.vector.tensor_reduce(
                    out=red[:, dve_col : dve_col + NARR],
                    in_=eo.rearrange("p (a f) -> p a f", f=F),
                    axis=AX.X,
                    op=OP.add,
                )
                dve_col += NARR

        # cross-partition combine of the DVE part -> outsb cols [0, dve_col)
        nc.gpsimd.tensor_reduce(
            out=outsb[0:1, 0:dve_col], in_=red[:, 0:dve_col], axis=AX.C, op=OP.add
        )
        nc.sync.dma_start(out=out_d[:, :], in_=outsb)

    nc.compile()
    return nc


def kernel(pred: np.ndarray, target: np.ndarray) -> np.ndarray:
    from concourse.bass_utils import run_bass_kernel_spmd

    if "nc" not in _CACHE:
        _CACHE["nc"] = _build_nc()
        _CACHE["R"] = _fit_R()
    nc = _CACHE["nc"]
    R = _CACHE["R"]

    pred = np.ascontiguousarray(np.asarray(pred, np.float32).reshape(B, N))
    target = np.ascontiguousarray(np.asarray(target, np.float32).reshape(B, N))
    in_maps = [
        {
            "pred": pred[i * SPC : (i + 1) * SPC],
            "target": target[i * SPC : (i + 1) * SPC],
        }
        for i in range(NCORES)
    ]
    res = run_bass_kernel_spmd(nc, in_maps, list(range(NCORES)))

    # assemble device basis sums: out col layout is k*NARR + a with
    # a in (pred_s0, targ_s0, pred_s1, targ_s1); DVE-reduced ks occupy
    # cols [0, (M-len(GP_KS))*NARR) in k-order, gpsimd ks sit at k*NARR+a.
    dve_ks = [k for k in range(M) if k not in GP_KS]
    Dp = np.zeros((B, M + 1))
    Dt = np.zeros((B, M + 1))
    Dp[:, M] = N
    Dt[:, M] = N
    for core in range(NCORES):
        raw = np.asarray(res.results[core]["dsums"], np.float64).reshape(-1)
        for s in range(SPC):
            b = core * SPC + s
            for ki, k in enumerate(dve_ks):
                Dp[b, k] = raw[ki * NARR + 2 * s]
                Dt[b, k] = raw[ki * NARR + 2 * s + 1]
            for k in GP_KS:
                Dp[b, k] = raw[k * NARR + 2 * s]
                Dt[b, k] = raw[k * NARR + 2 * s + 1]

    Sx = Dp @ R.T  # [B, 64] cumulative kernel sums
    Sy = Dt @ R.T
    js = np.arange(1, BINS + 1, dtype=np.float64)
    cdf_x = (Sx + js[None, :] * EPS) / (Sx[:, -1:] + BINS * EPS)
    cdf_y = (Sy + js[None, :] * EPS) / (Sy[:, -1:] + BINS * EPS)
    return np.float32(np.mean((cdf_x - cdf_y) ** 2))


# revision 8
# speedup vs baseline: 4.0545x; 1.2203x over previous
"""Trainium2 Bass kernel for nn_CDFVarianceLoss.

Math (per sample b, per tensor z in {pred[b], target[b]}, N = 65536):
    z' = (z - min z) / (max z - min z + 1e-6)
    h_j = sum_n exp(-(z'_n - c_j)^2 / (2*sigma^2)) + 1e-6,  c_j = j/63, j < 64
    cdf = cumsum(h / sum_j h)
    loss = mean_{b,j} (cdf_pred[b,j] - cdf_target[b,j])^2

Key identity: the cumulative kernel sums S_j = sum_n V_j(z'_n) with
V_j(z) = sum_{k<=j} exp(-alpha (z-c_k)^2) are smooth sigmoid-like functions
of z.  Each V_j is approximated (offline least squares, rel error on the
loss ~1e-5) by a linear combination of M=8 shifted erfs plus a constant:
    V_j(z) ~= sum_k R[j,k] erf((g_k - z)/s2) + R[j,M]
so the device only computes the M basis sums D_k = sum_n erf((g_k-z')/s2)
per array -- M activation passes over the natural [128, 512] layout with
scalar scale / per-partition-constant bias (the erf argument is affine in
z) -- and the host applies R in float64.  This removes the O(N*BINS)
exp/matmul grids of the direct approach entirely: ACT does N*M work.

Distribution: data-parallel over the batch -- 16 samples over 8 cores,
2 samples per core (4 arrays of 65536 per core: pred/target x 2 samples).

Per-core pipeline:
  - load 4 arrays natural [128, 512] fp32 into one [128, 2048] tile,
    one DMA queue per array (parallel)
  - min/max: two batched DVE segmented reduces [128, 4, 512] -> [128, 4]
    (carrying -min so the cross-lane finish can use max), one gpsimd
    cross-lane max [128, 8] -> [1, 8]
  - fixups on partition 0 -> (-zmin, 1/(zmax-zmin+eps)) x 4; PE ones-matmul
    broadcasts them to all partitions; DVE tensor_scalar normalizes
  - M x ACT Erf passes [128, 2048] (all 4 arrays at once), scale=-1/s2,
    bias=g_k/s2, bf16 outputs to rotating SBUF tiles
  - per-pass DVE reduce: one bf16 pair-fold (tensor_tensor add, 2x mode)
    then a segmented tensor_reduce [128, 4, 256] -> fp32 [128, 4]
  - PE transpose [128, 4M] -> [4M, 128], DVE reduce -> [4M, 1] basis sums
  - host: D @ R^T in fp64, then the exact eps/normalize/cumsum/mse tail
"""

import math

import numpy as np

B = 16
N = 65536
BINS = 64
SIGMA = 0.05
EPS = 1e-6
ALPHA = 0.5 / SIGMA**2  # 200.0
NCORES = 8
SPC = B // NCORES  # samples per core
NARR = 2 * SPC  # arrays per core: (pred, target) x samples
P = 128
F = N // P  # 512 natural free dim

# erf basis (designed offline; see module docstring)
M = 8
G_LO, G_HI = -0.14, 1.14
SIG_FIT = 0.14
S2 = SIG_FIT * math.sqrt(2.0)
G_PTS = [G_LO + (G_HI - G_LO) * k / (M - 1) for k in range(M)]

_CACHE = {}


def _fit_R():
    """Least-squares fit of V_j(z) in the erf basis + constant (fp64)."""
    nz = 40001
    zg = np.linspace(0.0, 1.0, nz)
    c = np.linspace(0.0, 1.0, BINS)
    K = np.exp(-ALPHA * (zg[None, :] - c[:, None]) ** 2)
    Vt = np.cumsum(K, axis=0)  # [64, nz]
    g = np.asarray(G_PTS)
    erf_v = np.vectorize(math.erf)
    Fb = np.concatenate(
        [erf_v((g[:, None] - zg[None, :]) / S2), np.ones((1, nz))], axis=0
    )  # [M+1, nz]
    w = np.ones(nz)
    w[zg < 0.02] = 3.0
    w[zg > 0.98] = 3.0
    Aw = Fb * w[None, :]
    Gm = Aw @ Fb.T
    Rhs = (Vt * w[None, :]) @ Fb.T
    return np.linalg.solve(Gm + 1e-10 * np.eye(M + 1), Rhs.T).T  # [64, M+1]


def _build_nc():
    import concourse.bass as bass  # noqa: F401
    import concourse.bacc as bacc
    import concourse.tile as tile
    from concourse import mybir
    from contextlib import ExitStack

    f32 = mybir.dt.float32
    bf16 = mybir.dt.bfloat16
    AX = mybir.AxisListType
    OP = mybir.AluOpType
    ACTF = mybir.ActivationFunctionType

    nc = bacc.Bacc()
    pred_d = nc.declare_dram_parameter("pred", [SPC, N], f32, isOutput=False)
    targ_d = nc.declare_dram_parameter("target", [SPC, N], f32, isOutput=False)
    out_d = nc.declare_dram_parameter("dsums", [NARR * M, 1], f32, isOutput=True)

    ones_row_np = np.ones((1, P), np.float32)
    bias_np = np.broadcast_to(
        np.asarray([g / S2 for g in G_PTS], np.float32)[None, :], (P, M)
    ).copy()
    ident_np = np.eye(P, dtype=np.float32)

    ones_d = nc.inline_tensor(ones_row_np, name="ones_row")
    bias_d = nc.inline_tensor(bias_np, name="erf_bias")
    ident_d = nc.inline_tensor(ident_np, name="ident")

    with tile.TileContext(nc) as tc, ExitStack() as ctx:
        singles = ctx.enter_context(tc.tile_pool(name="singles", bufs=1))
        nat = ctx.enter_context(tc.tile_pool(name="nat", bufs=1))
        eo_pool = ctx.enter_context(tc.tile_pool(name="eo", bufs=3))
        fold_pool = ctx.enter_context(tc.tile_pool(name="fold", bufs=3))
        small = ctx.enter_context(tc.tile_pool(name="small", bufs=2))
        ps_pool = ctx.enter_context(tc.tile_pool(name="ps", bufs=2, space="PSUM"))

        # natural input, all arrays side by side: [128, 4*512];
        # spread the loads over the three DMA-capable queues
        zn = nat.tile([P, NARR * F], f32, name="zn")
        srcs = [pred_d[0, :], targ_d[0, :], pred_d[1, :], targ_d[1, :]]
        qs = [nc.sync, nc.gpsimd, nc.scalar, nc.sync]
        for a, src in enumerate(srcs):
            qs[a].dma_start(
                out=zn[:, a * F : (a + 1) * F],
                in_=src.rearrange("(p f) -> p f", p=P),
            )

        ones_sb = singles.tile([1, P], f32)
        nc.gpsimd.dma_start(out=ones_sb, in_=ones_d[:, :])
        bias_sb = singles.tile([P, M], f32)
        nc.scalar.dma_start(out=bias_sb, in_=bias_d[:, :])
        ident_sb = singles.tile([P, P], f32)
        nc.scalar.dma_start(out=ident_sb, in_=ident_d[:, :])

        # batched per-array (-min, max): [128, 4] each (cross-lane gpsimd
        # reduce only supports add/avg/max, so carry -min and use max)
        mm = small.tile([P, 2 * NARR], f32, tag="mm")
        zn3 = zn.rearrange("p (a f) -> p a f", f=F)
        nc.vector.tensor_reduce(
            out=mm[:, 0:NARR], in_=zn3, axis=AX.X, op=OP.min, negate=True
        )
        nc.vector.tensor_reduce(
            out=mm[:, NARR : 2 * NARR], in_=zn3, axis=AX.X, op=OP.max
        )
        # cross-partition finish on gpsimd: [1, 8] = (-zmin x4 | zmax x4)
        mn_all = small.tile([1, 2 * NARR], f32, tag="mn")
        nc.gpsimd.tensor_reduce(out=mn_all, in_=mm, axis=AX.C, op=OP.max)

        # fixups on partition 0: nbsrc = [-zmin_a x4 | 1/(zmax-zmin+eps) x4]
        nbsrc = small.tile([1, 2 * NARR], f32, tag="nbsrc")
        rng = small.tile([1, NARR], f32, tag="rng")
        nc.vector.tensor_tensor(
            out=rng,
            in0=mn_all[0:1, 0:NARR],
            in1=mn_all[0:1, NARR : 2 * NARR],
            op=OP.add,
        )
        nc.vector.tensor_scalar_add(rng, rng, EPS)
        nc.vector.reciprocal(nbsrc[0:1, NARR : 2 * NARR], rng)
        nc.vector.tensor_copy(nbsrc[0:1, 0:NARR], mn_all[0:1, 0:NARR])

        # broadcast to all partitions with a ones-column matmul
        nbp = ps_pool.tile([P, 2 * NARR], f32, tag="ps")
        nc.tensor.matmul(nbp, ones_sb[:, :], nbsrc, start=True, stop=True)
        nb = small.tile([P, 2 * NARR], f32, tag="nb")
        nc.vector.tensor_copy(nb, nbp)

        # normalize each array in place -> z' in [0, 1]
        zc = nat.tile([P, NARR * F], f32, name="zc")
        for a in range(NARR):
            nc.vector.tensor_scalar(
                zc[:, a * F : (a + 1) * F],
                zn[:, a * F : (a + 1) * F],
                nb[:, a : a + 1],
                nb[:, NARR + a : NARR + a + 1],
                OP.add,
                OP.mult,
            )

        # erf spine; per-pass: one bf16 pair-fold (2x mode) + segmented reduce
        red = small.tile([P, M * NARR], f32, tag="red", name="red")
        H = F // 2
        for k in range(M):
            eo = eo_pool.tile([P, NARR * F], bf16, tag="eo")
            nc.scalar.activation(
                out=eo,
                in_=zc,
                func=ACTF.Erf,
                bias=bias_sb[:, k : k + 1],
                scale=float(-1.0 / S2),
            )
            eo3 = eo.rearrange("p (a f) -> p a f", f=F)
            fold = fold_pool.tile([P, NARR * H], bf16, tag="fold")
            fold3 = fold.rearrange("p (a h) -> p a h", h=H)
            nc.vector.tensor_tensor(
                out=fold3, in0=eo3[:, :, 0:H], in1=eo3[:, :, H:F], op=OP.add
            )
            nc.vector.tensor_reduce(
                out=red[:, k * NARR : (k + 1) * NARR],
                in_=fold3,
                axis=AX.X,
                op=OP.add,
            )

        # cross-partition combine: PE transpose + DVE reduce
        redT_p = ps_pool.tile([M * NARR, P], f32, tag="ps")
        nc.tensor.transpose(redT_p, red, ident_sb[:, :])
        redT = small.tile([M * NARR, P], f32, tag="redT")
        nc.vector.tensor_copy(redT, redT_p)
        dvec = small.tile([M * NARR, 1], f32, tag="dvec")
        nc.vector.tensor_reduce(out=dvec, in_=redT, axis=AX.X, op=OP.add)
        nc.scalar.dma_start(out=out_d[:, :], in_=dvec)

    nc.compile()
    return nc


def kernel(pred: np.ndarray, target: np.ndarray) -> np.ndarray:
    from concourse.bass_utils import run_bass_kernel_spmd

    if "nc" not in _CACHE:
        _CACHE["nc"] = _build_nc()
        _CACHE["R"] = _fit_R()
    nc = _CACHE["nc"]
    R = _CACHE["R"]

    pred = np.ascontiguousarray(np.asarray(pred, np.float32).reshape(B, N))
    target = np.ascontiguousarray(np.asarray(target, np.float32).reshape(B, N))
    in_maps = [
        {
            "pred": pred[i * SPC : (i + 1) * SPC],
            "target": target[i * SPC : (i + 1) * SPC],
        }
        for i in range(NCORES)
    ]
    res = run_bass_kernel_spmd(nc, in_maps, list(range(NCORES)))

    # device dsums col layout: k*NARR + a, a in (pred_s0, targ_s0,
    # pred_s1, targ_s1)
    Dp = np.zeros((B, M + 1))
    Dt = np.zeros((B, M + 1))
    Dp[:, M] = N
    Dt[:, M] = N
    for core in range(NCORES):
        raw = np.asarray(res.results[core]["dsums"], np.float64).reshape(-1)
        for s in range(SPC):
            b = core * SPC + s
            for k in range(M):
                Dp[b, k] = raw[k * NARR + 2 * s]
                Dt[b, k] = raw[k * NARR + 2 * s + 1]

    Sx = Dp @ R.T  # [B, 64] cumulative kernel sums
    Sy = Dt @ R.T
    js = np.arange(1, BINS + 1, dtype=np.float64)
    cdf_x = (Sx + js[None, :] * EPS) / (Sx[:, -1:] + BINS * EPS)
    cdf_y = (Sy + js[None, :] * EPS) / (Sy[:, -1:] + BINS * EPS)
    return np.float32(np.mean((cdf_x - cdf_y) ** 2))
